# revision 4
# baseline (speedup 1.0000x reference)
"""Trainium2 Bass kernel for GemmaAttention (B=2, S=2048, HID=1024, NH=4, HD=256).

Sharding: 8 cores = batch(2) x heads(4). Each core computes one (b, h):
  q/k/v projections for its head, RoPE, attention, and a partial output
  projection [S, HID]; the host sums the 4 per-head partials per batch.

Design notes (v3 — fused pipeline, f32r):
  - Single fused PE stream: projections, attention and the output projection
    interleave so the tensor engine never idles (idle resets the PE p-state
    from 2.4GHz back to 1.2GHz). For causal masks proj(r+1) is interleaved
    after attn(r-1); for full attention all projections are emitted first.
  - All matmul operands are float32r: fp32 data read at full PE rate when the
    moving free dim >= 256 (FP22 multiply, fp32 accumulate) — same speed as
    bf16 at these tile shapes, ~10x better precision.
  - Scores are computed transposed, ST[j, i] = (q_i . k_j): exp needs no max
    subtraction, l[i] = sum_j P[j,i] is a ones-stationary matmul, and P^T is
    exactly the PV moving operand (no transposes anywhere).
  - Mask modes compiled on demand:
      nomask : mask == 0 -> full attention, no mask work at all
      causal : standard tril(-1e9) mask -> lower-triangle tiles only, binary
               multiplicative mask on diagonal tiles after exp; diagonal
               tiles are issued FIRST per chunk so the extra DVE mask latency
               hides in the score-pipeline prologue
      generic: arbitrary additive mask -> host precomputes exp(mask^T) in
               bf16; streamed in and applied multiplicatively after exp
  - Engine placement: exp / psum drains / (1/l) output scaling on ScalarE,
    RoPE on VectorE (ordered to release PSUM banks early), matmuls on PE.
"""

import sys

sys.path.insert(0, "/opt/trn_rl_repo")

from collections import deque

import numpy as np
import ml_dtypes

import concourse.bacc as bacc
import concourse.bass as bass
import concourse.mybir as mybir
import concourse.tile as tile
from concourse.bass_utils import run_bass_kernel_spmd


def _ensure_ntff_hook():
    """This image's ``antenv`` lacks ``axon_hooks`` (bass_utils imports it for
    trace=True). Inject an equivalent module driving NTFF profiling via the
    libaxon C ABI (mirrors trn_agent_boot._ntff_profile_via_ctypes)."""
    import types, ctypes, contextlib, os

    if "antenv.axon_hooks" in sys.modules:
        return
    so_path = "/opt/axon/libaxon_pjrt.so"
    hook = None
    if os.path.exists(so_path):
        lib = ctypes.CDLL(so_path)
        if hasattr(lib, "axon_start_nrt_profile"):
            lib.axon_start_nrt_profile.argtypes = [
                ctypes.POINTER(ctypes.c_int64),
                ctypes.c_size_t,
            ]
            lib.axon_start_nrt_profile.restype = ctypes.c_int64
            lib.axon_stop_nrt_profile.argtypes = [ctypes.c_char_p]
            lib.axon_stop_nrt_profile.restype = ctypes.c_int64

            @contextlib.contextmanager
            def _hook(output_dir, device_ids):
                import jax

                jax.devices()
                if device_ids:
                    ids = (ctypes.c_int64 * len(device_ids))(*device_ids)
                    rc = lib.axon_start_nrt_profile(ids, len(device_ids))
                else:
                    rc = lib.axon_start_nrt_profile(None, 0)
                if rc != 0:
                    raise RuntimeError(f"axon_start_nrt_profile rc={rc}")
                try:
                    yield
                finally:
                    n = lib.axon_stop_nrt_profile(str(output_dir).encode())
                    if n < 0:
                        raise RuntimeError(f"axon_stop_nrt_profile rc={n}")
                    print(f"profile: {n} file(s) written to {output_dir}")

            hook = _hook

    mod = types.ModuleType("antenv.axon_hooks")
    _state = {"hook": hook}
    mod.set_axon_ntff_profile_hook = lambda h: _state.__setitem__("hook", h)
    mod.get_axon_ntff_profile_hook = lambda: _state["hook"]
    sys.modules["antenv.axon_hooks"] = mod
    import antenv

    antenv.axon_hooks = mod


B, S, HID = 2, 2048, 1024
NH, HD = 4, 256
SCALE = HD**-0.5
P = 128
CH = 512          # i-chunk width (and matmul moving free-dim)
NSC = S // CH     # 4 i-chunks
NJT = S // P      # 16 j-tiles
KT = HID // P     # 8 contraction tiles for projections
ND = CH // P      # 4 i-subtiles per chunk

_cache = {}
F32R = mybir.dt.float32r
BF16 = mybir.dt.bfloat16
NPBF16 = ml_dtypes.bfloat16


def build_nc(mode):
    """Emit the single-core program (SPMD: all 8 cores run this)."""
    assert mode in ("nomask", "causal", "generic")
    nc = bacc.Bacc()
    f32 = mybir.dt.float32
    Exp = mybir.ActivationFunctionType.Exp

    xT = nc.declare_dram_parameter("xT", [KT, P, S], F32R, isOutput=False)
    wq = nc.declare_dram_parameter("wq", [P, KT, HD], F32R, isOutput=False)
    wk = nc.declare_dram_parameter("wk", [P, KT, HD], F32R, isOutput=False)
    wv = nc.declare_dram_parameter("wv", [P, KT, HD], F32R, isOutput=False)
    wo = nc.declare_dram_parameter("wo", [P, HD // P, HID], F32R, isOutput=False)
    frT = nc.declare_dram_parameter("frT", [P, S], f32, isOutput=False)
    fiT = nc.declare_dram_parameter("fiT", [P, S], f32, isOutput=False)
    ones = nc.declare_dram_parameter("ones", [P, 2], F32R, isOutput=False)
    ones2 = nc.declare_dram_parameter("ones2", [1, 2], F32R, isOutput=False)
    if mode == "causal":
        mk = nc.declare_dram_parameter("mk", [P, ND, CH], f32, isOutput=False)
    elif mode == "generic":
        mke = nc.declare_dram_parameter("mke", [P, NJT, S], BF16, isOutput=False)
    out = nc.declare_dram_parameter("out", [S, HID], f32, isOutput=True)

    with tile.TileContext(nc) as tc:
        with (
            tc.tile_pool(name="consts", bufs=1) as consts,
            tc.tile_pool(name="xp", bufs=2) as xp,
            tc.tile_pool(name="qk", bufs=1) as qk,
            tc.tile_pool(name="rst", bufs=2) as rst,
            tc.tile_pool(name="pw", bufs=3) as pw,
            tc.tile_pool(name="aw", bufs=2) as aw,
            tc.tile_pool(name="lw", bufs=2) as lw,
            tc.tile_pool(name="obp", bufs=3) as obp,
            tc.tile_pool(name="psp", bufs=1, space="PSUM") as psp,
        ):
            # ---------------- constant + input loads ----------------
            # Order matters: wq + xT chunk 0 first so the PE starts ASAP.
            wq_sb = consts.tile([P, KT, HD], F32R)
            nc.sync.dma_start(out=wq_sb, in_=wq[:])

            def load_xc(c):
                xc = xp.tile([P, KT, CH], F32R, tag="xc", name="xc")
                for kt in range(KT):
                    nc.sync.dma_start(
                        out=xc[:, kt, :], in_=xT[kt, :, c * CH : (c + 1) * CH]
                    )
                return xc

            xcs = [load_xc(0)]
            wk_sb = consts.tile([P, KT, HD], F32R)
            nc.sync.dma_start(out=wk_sb, in_=wk[:])
            frT_sb = consts.tile([P, S], f32)
            fiT_sb = consts.tile([P, S], f32)
            nc.sync.dma_start(out=frT_sb, in_=frT[:])
            nc.sync.dma_start(out=fiT_sb, in_=fiT[:])
            wv_sb = consts.tile([P, KT, HD], F32R)
            nc.sync.dma_start(out=wv_sb, in_=wv[:])
            xcs.append(load_xc(1))
            wo_sb = consts.tile([P, HD // P, HID], F32R)
            nc.sync.dma_start(out=wo_sb, in_=wo[:])
            if mode == "causal":
                mk_sb = consts.tile([P, ND, CH], f32)
                nc.sync.dma_start(out=mk_sb, in_=mk[:])
            ones_sb = consts.tile([P, 2], F32R)
            nc.sync.dma_start(out=ones_sb, in_=ones[:])
            ones2_sb = consts.tile([1, 2], F32R)
            nc.sync.dma_start(out=ones2_sb, in_=ones2[:])

            # persistent activations
            qrT_sb = qk.tile([P, HD // P, S], F32R)   # rope'd qT (d on partitions)
            krT_sb = qk.tile([P, HD // P, S], F32R)
            v_sb = qk.tile([P, NJT, HD], F32R)        # v[j, e] per j-tile

            # ---------------- building blocks ----------------
            def proj(c):
                """q/k projections + fused rope for i-chunk c; v for its 4
                i-tiles. RoPE op order releases ps0 after 2 DVE ops so the
                "st" psum ring (3 bufs) never stalls the next projection."""
                xc = xcs[c]
                cs = slice(c * CH, (c + 1) * CH)
                for wsb, dst in ((wq_sb, qrT_sb), (wk_sb, krT_sb)):
                    ps0 = psp.tile([P, CH], f32, tag="st", bufs=3, name="ps0")
                    ps1 = psp.tile([P, CH], f32, tag="st", bufs=3, name="ps1")
                    for m, ps in ((0, ps0), (1, ps1)):
                        for kt in range(KT):
                            nc.tensor.matmul(
                                ps,
                                wsb[:, kt, m * P : (m + 1) * P],
                                xc[:, kt, :],
                                start=(kt == 0),
                                stop=(kt == KT - 1),
                            )
                    fr = frT_sb[:, cs]
                    fi = fiT_sb[:, cs]
                    ta = rst.tile([P, CH], f32, tag="ta", bufs=2, name="ta")
                    tb = rst.tile([P, CH], f32, tag="tb", bufs=2, name="tb")
                    # dst0 = ps0*fr - ps1*fi ; dst1 = ps1*fr + ps0*fi
                    nc.vector.tensor_mul(dst[:, 0, cs], ps0, fr)
                    nc.vector.tensor_mul(tb, ps0, fi)
                    nc.vector.tensor_mul(ta, ps1, fi)
                    nc.vector.tensor_mul(dst[:, 1, cs], ps1, fr)
                    nc.vector.tensor_sub(dst[:, 0, cs], dst[:, 0, cs], ta)
                    nc.vector.tensor_add(dst[:, 1, cs], dst[:, 1, cs], tb)
                for sl in range(ND):
                    st = ND * c + sl
                    psv = psp.tile([P, HD], f32, tag="o", bufs=2, name="psv")
                    for kt in range(KT):
                        nc.tensor.matmul(
                            psv,
                            xc[:, kt, sl * P : (sl + 1) * P],
                            wv_sb[:, kt, :],
                            start=(kt == 0),
                            stop=(kt == KT - 1),
                        )
                    nc.scalar.copy(v_sb[:, st, :], psv)

            def fin_setup(l_sb):
                """Transpose l to partitions (tiny matmuls) + reciprocal."""
                rl_ps = psp.tile([P, 2 * ND], f32, tag="o", bufs=2, name="rlps")
                for i in range(ND):
                    nc.tensor.matmul(
                        rl_ps[:, 2 * i : 2 * i + 2],
                        l_sb[:, i * P : (i + 1) * P],
                        ones2_sb,
                        start=True,
                        stop=True,
                    )
                rl_sb = lw.tile([P, 2 * ND], f32, tag="rl", bufs=2, name="rlsb")
                nc.vector.reciprocal(rl_sb, rl_ps)
                return rl_sb

            def fin_isub(c, attn_sb, rl_sb, isub):
                """Output projection + 1/l scaling + store for one i-subtile."""
                ob = obp.tile([P, HID], f32, tag="ob", bufs=3, name="ob")
                for fc in range(HID // CH):
                    ops = psp.tile([P, CH], f32, tag="o", bufs=2, name="ops")
                    for et in range(HD // P):
                        nc.tensor.matmul(
                            ops,
                            attn_sb[:, et, isub * P : (isub + 1) * P],
                            wo_sb[:, et, fc * CH : (fc + 1) * CH],
                            start=(et == 0),
                            stop=(et == HD // P - 1),
                        )
                    nc.scalar.mul(
                        ob[:, fc * CH : (fc + 1) * CH],
                        ops,
                        rl_sb[:, 2 * isub : 2 * isub + 1],
                    )
                nc.sync.dma_start(
                    out=out[c * CH + isub * P : c * CH + (isub + 1) * P, :],
                    in_=ob,
                )

            def attn(r, pending):
                """Attention for i-chunk r (transposed-scores softmax).
                pending = (c, attn_sb, rl_sb) of chunk c=r-1 whose output
                projection is interleaved at pipeline steps 1..4."""
                if mode == "causal":
                    # diagonal (masked) tiles first: their extra DVE mask
                    # latency hides in the 2-deep score-pipeline prologue
                    js = list(range(ND * r, ND * r + ND)) + list(range(ND * r))
                else:
                    js = list(range(NJT))
                jmax = len(js)
                ics = slice(r * CH, (r + 1) * CH)
                attn_ps = psp.tile([P, HD // P, CH], f32, tag="big", bufs=1, name="at")
                l_ps = psp.tile([1, CH], f32, tag="l", bufs=1, name="l")

                def mk_p(j):
                    stp = psp.tile([P, CH], f32, tag="st", bufs=3, name="stp")
                    for dt in range(HD // P):
                        nc.tensor.matmul(
                            stp,
                            krT_sb[:, dt, j * P : (j + 1) * P],
                            qrT_sb[:, dt, ics],
                            start=(dt == 0),
                            stop=(dt == HD // P - 1),
                        )
                    p = pw.tile([P, CH], F32R, tag="p", bufs=3, name="p")
                    nc.scalar.activation(p, stp, Exp)
                    if mode == "causal" and j >= ND * r:
                        nc.vector.tensor_mul(p, p, mk_sb[:, j - ND * r, :])
                    elif mode == "generic":
                        me = pw.tile([P, CH], BF16, tag="me", bufs=3, name="me")
                        nc.sync.dma_start(out=me, in_=mke[:, j, ics])
                        nc.vector.tensor_mul(p, p, me)
                    return p

                pq = deque()
                pq.append(mk_p(js[0]))
                pq.append(mk_p(js[1]))
                for t in range(jmax):
                    if t + 2 < jmax:
                        pq.append(mk_p(js[t + 2]))
                    p = pq.popleft()
                    j = js[t]
                    first, last = t == 0, t == jmax - 1
                    for et in range(HD // P):
                        nc.tensor.matmul(
                            attn_ps[:, et, :],
                            v_sb[:, j, et * P : (et + 1) * P],
                            p,
                            start=first,
                            stop=last,
                        )
                    nc.tensor.matmul(
                        l_ps, ones_sb[:, 0:1], p, start=first, stop=last
                    )
                    if pending is not None and 1 <= t <= ND:
                        fin_isub(pending[0], pending[1], pending[2], t - 1)
                # drain psums (frees banks for the next chunk)
                attn_sb = aw.tile([P, HD // P, CH], F32R, tag="at", bufs=2, name="atsb")
                nc.scalar.copy(attn_sb, attn_ps)
                l_sb = lw.tile([1, CH], F32R, tag="l", bufs=2, name="lsb")
                nc.vector.tensor_copy(l_sb, l_ps)
                return (r, attn_sb, l_sb)

            # ---------------- fused main pipeline ----------------
            # Causal: attn(r) only needs k/v chunks <= r, so proj(r+1) can be
            # interleaved after attn(r-1). Full attention (nomask/generic):
            # attn(0) reads ALL k/v chunks, so every projection must be
            # emitted first (still one continuous PE stream).
            proj(0)
            if mode != "causal":
                for c in range(1, NSC):
                    xcs.append(load_xc(c + 1) if c + 1 < NSC else None)
                    proj(c)
                # reorder: load_xc must come before proj uses it
            drained = None
            for r in range(NSC):
                if mode == "causal" and r + 1 < NSC:
                    proj(r + 1)
                    if r + 2 < NSC:
                        xcs.append(load_xc(r + 2))
                pend = None
                if drained is not None:
                    rl_sb = fin_setup(drained[2])
                    pend = (drained[0], drained[1], rl_sb)
                drained = attn(r, pend)
            rl_sb = fin_setup(drained[2])
            for isub in range(ND):
                fin_isub(drained[0], drained[1], rl_sb, isub)

    nc.compile()
    return nc


def _perm():
    return np.concatenate([np.arange(0, HD, 2), np.arange(1, HD, 2)])


def make_core_inputs(hidden_states, freqs_real, freqs_imag, mask, W_qkv, W_o, mode):
    """Host-side shard + relayout (free). Returns 8 in_maps (core = b*NH + h)."""
    perm = _perm()
    frT = np.ascontiguousarray(freqs_real.T.astype(np.float32))
    fiT = np.ascontiguousarray(freqs_imag.T.astype(np.float32))
    extras = {}
    if mode == "causal":
        r = np.arange(P)[:, None, None]
        o = np.arange(ND)[None, :, None]
        cc = np.arange(CH)[None, None, :]
        extras["mk"] = np.ascontiguousarray((cc >= r + P * o).astype(np.float32))
    elif mode == "generic":
        # exp(mask^T)[j, i] reshaped to [p, jt, i]
        m = np.exp(np.asarray(mask[0, 0], dtype=np.float64).T)  # [j, i]
        m = m.reshape(NJT, P, S).transpose(1, 0, 2)
        extras["mke"] = np.ascontiguousarray(m).astype(NPBF16)
    in_maps = []
    for b in range(B):
        xTr = np.ascontiguousarray(
            hidden_states[b].T.astype(np.float32).reshape(KT, P, S)
        )
        for h in range(NH):
            wq_h = (W_qkv[h * HD : (h + 1) * HD, :][perm, :] * SCALE).T
            wk_h = W_qkv[HID + h * HD : HID + (h + 1) * HD, :][perm, :].T
            wv_h = W_qkv[2 * HID + h * HD : 2 * HID + (h + 1) * HD, :].T
            wo_h = W_o[:, h * HD : (h + 1) * HD].T
            in_maps.append(
                {
                    "xT": xTr,
                    "wq": np.ascontiguousarray(
                        wq_h.reshape(KT, P, HD).transpose(1, 0, 2).astype(np.float32)
                    ),
                    "wk": np.ascontiguousarray(
                        wk_h.reshape(KT, P, HD).transpose(1, 0, 2).astype(np.float32)
                    ),
                    "wv": np.ascontiguousarray(
                        wv_h.reshape(KT, P, HD).transpose(1, 0, 2).astype(np.float32)
                    ),
                    "wo": np.ascontiguousarray(
                        wo_h.reshape(HD // P, P, HID)
                        .transpose(1, 0, 2)
                        .astype(np.float32)
                    ),
                    "frT": frT,
                    "fiT": fiT,
                    "ones": np.ones((P, 2), dtype=np.float32),
                    "ones2": np.ones((1, 2), dtype=np.float32),
                    **extras,
                }
            )
    return in_maps


def _mask_mode(mask):
    m = np.asarray(mask)
    if m.shape != (1, 1, S, S):
        return "generic"
    if not np.any(m):
        return "nomask"
    causal = np.tril(np.ones((S, S), dtype=bool))
    expect = np.where(causal, np.float32(0.0), np.float32(-1e9))
    if np.array_equal(m[0, 0], expect):
        return "causal"
    return "generic"


def kernel(hidden_states, freqs_real, freqs_imag, mask, W_qkv, W_o, _trace=False):
    hidden_states = np.asarray(hidden_states, dtype=np.float32)
    freqs_real = np.asarray(freqs_real, dtype=np.float32)
    freqs_imag = np.asarray(freqs_imag, dtype=np.float32)
    mask = np.asarray(mask)
    W_qkv = np.asarray(W_qkv, dtype=np.float32)
    W_o = np.asarray(W_o, dtype=np.float32)

    if _trace:
        _ensure_ntff_hook()
    mode = _mask_mode(mask)
    if mode not in _cache:
        _cache[mode] = build_nc(mode)
    nc = _cache[mode]
    in_maps = make_core_inputs(
        hidden_states, freqs_real, freqs_imag, mask, W_qkv, W_o, mode
    )
    res = run_bass_kernel_spmd(nc, in_maps, list(range(B * NH)), trace=_trace)
    outs = [res.results[i]["out"] for i in range(B * NH)]
    full = np.zeros((B, S, HID), dtype=np.float32)
    for b in range(B):
        for h in range(NH):
            full[b] += outs[b * NH + h]
    if _trace:
        return full, res
    return full


# revision 8
# speedup vs baseline: 1.1122x; 1.1122x over previous
"""Trainium2 Bass kernel for GemmaAttention (B=2, S=2048, HID=1024, NH=4, HD=256).

Sharding: 8 cores = batch(2) x heads(4). Each core computes one (b, h):
  q/k/v projections for its head, RoPE, attention, and a partial output
  projection [S, HID]; the host sums the 4 per-head partials per batch.

Design notes (v3 — fused pipeline, f32r):
  - Single fused PE stream: projections, attention and the output projection
    interleave so the tensor engine never idles (idle resets the PE p-state
    from 2.4GHz back to 1.2GHz). For causal masks proj(r+1) is interleaved
    after attn(r-1); for full attention all projections are emitted first.
  - All matmul operands are float32r: fp32 data read at full PE rate when the
    moving free dim >= 256 (FP22 multiply, fp32 accumulate) — same speed as
    bf16 at these tile shapes, ~10x better precision.
  - Scores are computed transposed, ST[j, i] = (q_i . k_j): exp needs no max
    subtraction, l[i] = sum_j P[j,i] is a ones-stationary matmul, and P^T is
    exactly the PV moving operand (no transposes anywhere).
  - Mask modes compiled on demand:
      nomask : mask == 0 -> full attention, no mask work at all
      causal : standard tril(-1e9) mask -> lower-triangle tiles only, binary
               multiplicative mask on diagonal tiles after exp; diagonal
               tiles are issued FIRST per chunk so the extra DVE mask latency
               hides in the score-pipeline prologue
      generic: arbitrary additive mask -> host precomputes exp(mask^T) in
               bf16; streamed in and applied multiplicatively after exp
  - Engine placement: exp / psum drains / (1/l) output scaling on ScalarE,
    RoPE on VectorE (ordered to release PSUM banks early), matmuls on PE.
"""

import sys

sys.path.insert(0, "/opt/trn_rl_repo")

from collections import deque

import numpy as np
import ml_dtypes

import concourse.bacc as bacc
import concourse.bass as bass
import concourse.mybir as mybir
import concourse.tile as tile
from concourse.bass_utils import run_bass_kernel_spmd


def _ensure_ntff_hook():
    """This image's ``antenv`` lacks ``axon_hooks`` (bass_utils imports it for
    trace=True). Inject an equivalent module driving NTFF profiling via the
    libaxon C ABI (mirrors trn_agent_boot._ntff_profile_via_ctypes)."""
    import types, ctypes, contextlib, os

    if "antenv.axon_hooks" in sys.modules:
        return
    so_path = "/opt/axon/libaxon_pjrt.so"
    hook = None
    if os.path.exists(so_path):
        lib = ctypes.CDLL(so_path)
        if hasattr(lib, "axon_start_nrt_profile"):
            lib.axon_start_nrt_profile.argtypes = [
                ctypes.POINTER(ctypes.c_int64),
                ctypes.c_size_t,
            ]
            lib.axon_start_nrt_profile.restype = ctypes.c_int64
            lib.axon_stop_nrt_profile.argtypes = [ctypes.c_char_p]
            lib.axon_stop_nrt_profile.restype = ctypes.c_int64

            @contextlib.contextmanager
            def _hook(output_dir, device_ids):
                import jax

                jax.devices()
                if device_ids:
                    ids = (ctypes.c_int64 * len(device_ids))(*device_ids)
                    rc = lib.axon_start_nrt_profile(ids, len(device_ids))
                else:
                    rc = lib.axon_start_nrt_profile(None, 0)
                if rc != 0:
                    raise RuntimeError(f"axon_start_nrt_profile rc={rc}")
                try:
                    yield
                finally:
                    n = lib.axon_stop_nrt_profile(str(output_dir).encode())
                    if n < 0:
                        raise RuntimeError(f"axon_stop_nrt_profile rc={n}")
                    print(f"profile: {n} file(s) written to {output_dir}")

            hook = _hook

    mod = types.ModuleType("antenv.axon_hooks")
    _state = {"hook": hook}
    mod.set_axon_ntff_profile_hook = lambda h: _state.__setitem__("hook", h)
    mod.get_axon_ntff_profile_hook = lambda: _state["hook"]
    sys.modules["antenv.axon_hooks"] = mod
    import antenv

    antenv.axon_hooks = mod


B, S, HID = 2, 2048, 1024
NH, HD = 4, 256
SCALE = HD**-0.5
P = 128
CH = 512          # i-chunk width (and matmul moving free-dim)
NSC = S // CH     # 4 i-chunks
NJT = S // P      # 16 j-tiles
KT = HID // P     # 8 contraction tiles for projections
ND = CH // P      # 4 i-subtiles per chunk

_cache = {}
F32R = mybir.dt.float32r
FP16 = mybir.dt.float16
BF16 = mybir.dt.bfloat16
NPBF16 = ml_dtypes.bfloat16


def build_nc(mode):
    """Emit the single-core program (SPMD: all 8 cores run this)."""
    assert mode in ("nomask", "causal", "generic")
    nc = bacc.Bacc()
    f32 = mybir.dt.float32
    Exp = mybir.ActivationFunctionType.Exp

    xT = nc.declare_dram_parameter("xT", [KT, P, S], FP16, isOutput=False)
    wq = nc.declare_dram_parameter("wq", [P, KT, HD], FP16, isOutput=False)
    wk = nc.declare_dram_parameter("wk", [P, KT, HD], FP16, isOutput=False)
    wv = nc.declare_dram_parameter("wv", [P, KT, HD], FP16, isOutput=False)
    wo = nc.declare_dram_parameter("wo", [P, HD // P, HID], FP16, isOutput=False)
    frT = nc.declare_dram_parameter("frT", [P, S], f32, isOutput=False)
    fiT = nc.declare_dram_parameter("fiT", [P, S], f32, isOutput=False)
    ones = nc.declare_dram_parameter("ones", [P, 4], FP16, isOutput=False)
    ones2 = nc.declare_dram_parameter("ones2", [1, 2], F32R, isOutput=False)
    if mode == "causal":
        mk = nc.declare_dram_parameter("mk", [P, ND, CH], FP16, isOutput=False)
    elif mode == "generic":
        mke = nc.declare_dram_parameter("mke", [P, NJT, S], BF16, isOutput=False)
    out = nc.declare_dram_parameter("out", [S, HID], f32, isOutput=True)

    with tile.TileContext(nc) as tc:
        with (
            tc.tile_pool(name="consts", bufs=1) as consts,
            tc.tile_pool(name="xp", bufs=2) as xp,
            tc.tile_pool(name="qk", bufs=1) as qk,
            tc.tile_pool(name="rst", bufs=2) as rst,
            tc.tile_pool(name="pw", bufs=3) as pw,
            tc.tile_pool(name="aw", bufs=2) as aw,
            tc.tile_pool(name="lw", bufs=2) as lw,
            tc.tile_pool(name="obp", bufs=3) as obp,
            tc.tile_pool(name="psp", bufs=1, space="PSUM") as psp,
        ):
            # ---------------- constant + input loads ----------------
            # Order matters: the first projection matmul only needs wq[kt=0]
            # and xc0[kt=0], so weights are loaded per-kt interleaved with the
            # xT chunk tiles, and freqs per-chunk, to start the PE ASAP.
            wq_sb = consts.tile([P, KT, HD], FP16)
            wk_sb = consts.tile([P, KT, HD], FP16)
            frT_sb = consts.tile([P, S], f32)
            fiT_sb = consts.tile([P, S], f32)

            def load_xc(c):
                xc = xp.tile([P, KT, CH], FP16, tag="xc", name="xc")
                for kt in range(KT):
                    nc.sync.dma_start(
                        out=xc[:, kt, :], in_=xT[kt, :, c * CH : (c + 1) * CH]
                    )
                return xc

            # bulk stream (weights + x chunks + stores) on the sync HWDGE
            # ring; small constants (freqs/mask/ones) on the independent
            # scalar-engine HWDGE ring so they don't queue behind the bulk.
            xc0 = xp.tile([P, KT, CH], FP16, tag="xc", name="xc")
            for kt in range(KT):
                nc.sync.dma_start(out=wq_sb[:, kt, :], in_=wq[:, kt, :])
                nc.sync.dma_start(out=wk_sb[:, kt, :], in_=wk[:, kt, :])
                nc.sync.dma_start(out=xc0[:, kt, :], in_=xT[kt, :, 0:CH])
            nc.scalar.dma_start(out=frT_sb, in_=frT[:])
            nc.scalar.dma_start(out=fiT_sb, in_=fiT[:])

            def load_xc(c):
                xc = xp.tile([P, KT, CH], FP16, tag="xc", name="xc")
                for kt in range(KT):
                    nc.sync.dma_start(
                        out=xc[:, kt, :], in_=xT[kt, :, c * CH : (c + 1) * CH]
                    )
                return xc

            wv_sb = consts.tile([P, KT, HD], FP16)
            nc.sync.dma_start(out=wv_sb, in_=wv[:])
            xcs = [xc0, load_xc(1)]
            wo_sb = consts.tile([P, HD // P, HID], FP16)
            nc.sync.dma_start(out=wo_sb, in_=wo[:])
            if mode == "causal":
                mk_sb = consts.tile([P, ND, CH], FP16)
                nc.scalar.dma_start(out=mk_sb, in_=mk[:])
            ones_sb = consts.tile([P, 4], FP16)
            nc.scalar.dma_start(out=ones_sb, in_=ones[:])
            ones2_sb = consts.tile([1, 2], F32R)
            nc.scalar.dma_start(out=ones2_sb, in_=ones2[:])

            # persistent activations
            qrT_sb = qk.tile([P, HD // P, S], FP16)   # rope'd qT (d on partitions)
            krT_sb = qk.tile([P, HD // P, S], FP16)
            v_sb = qk.tile([P, NJT, HD], FP16)        # v[j, e] per j-tile

            # ---------------- building blocks ----------------
            def proj(c):
                """q/k projections + fused rope for i-chunk c; v for its 4
                i-tiles. RoPE op order releases ps0 after 2 DVE ops so the
                "st" psum ring (3 bufs) never stalls the next projection."""
                xc = xcs[c]
                cs = slice(c * CH, (c + 1) * CH)
                for wsb, dst in ((wq_sb, qrT_sb), (wk_sb, krT_sb)):
                    ps0 = psp.tile([P, CH], f32, tag="st", bufs=3, name="ps0")
                    ps1 = psp.tile([P, CH], f32, tag="st", bufs=3, name="ps1")
                    for m, ps in ((0, ps0), (1, ps1)):
                        for kt in range(KT):
                            nc.tensor.matmul(
                                ps,
                                wsb[:, kt, m * P : (m + 1) * P],
                                xc[:, kt, :],
                                start=(kt == 0),
                                stop=(kt == KT - 1),
                            )
                    fr = frT_sb[:, cs]
                    fi = fiT_sb[:, cs]
                    ta = rst.tile([P, CH], f32, tag="ta", bufs=2, name="ta")
                    tb = rst.tile([P, CH], f32, tag="tb", bufs=2, name="tb")
                    # dst0 = ps0*fr - ps1*fi ; dst1 = ps1*fr + ps0*fi
                    nc.vector.tensor_mul(dst[:, 0, cs], ps0, fr)
                    nc.vector.tensor_mul(tb, ps0, fi)
                    nc.vector.tensor_mul(ta, ps1, fi)
                    nc.vector.tensor_mul(dst[:, 1, cs], ps1, fr)
                    nc.vector.tensor_sub(dst[:, 0, cs], dst[:, 0, cs], ta)
                    nc.vector.tensor_add(dst[:, 1, cs], dst[:, 1, cs], tb)
                for sl in range(ND):
                    st = ND * c + sl
                    psv = psp.tile([P, HD], f32, tag="o", bufs=2, name="psv")
                    for kt in range(KT):
                        nc.tensor.matmul(
                            psv,
                            xc[:, kt, sl * P : (sl + 1) * P],
                            wv_sb[:, kt, :],
                            start=(kt == 0),
                            stop=(kt == KT - 1),
                        )
                    nc.scalar.copy(v_sb[:, st, :], psv)

            def fin_setup(l_sb):
                """Transpose l to partitions (tiny matmuls) + reciprocal."""
                rl_ps = psp.tile([P, 2 * ND], f32, tag="o", bufs=2, name="rlps")
                for i in range(ND):
                    nc.tensor.matmul(
                        rl_ps[:, 2 * i : 2 * i + 2],
                        l_sb[:, i * P : (i + 1) * P],
                        ones2_sb,
                        start=True,
                        stop=True,
                    )
                rl_sb = lw.tile([P, 2 * ND], f32, tag="rl", bufs=2, name="rlsb")
                nc.vector.reciprocal(rl_sb, rl_ps)
                return rl_sb

            def fin_isub(c, attn_sb, rl_sb, isub):
                """Output projection + 1/l scaling + store for one i-subtile."""
                ob = obp.tile([P, HID], f32, tag="ob", bufs=3, name="ob")
                for fc in range(HID // CH):
                    ops = psp.tile([P, CH], f32, tag="o", bufs=2, name="ops")
                    for et in range(HD // P):
                        nc.tensor.matmul(
                            ops,
                            attn_sb[:, et, isub * P : (isub + 1) * P],
                            wo_sb[:, et, fc * CH : (fc + 1) * CH],
                            start=(et == 0),
                            stop=(et == HD // P - 1),
                        )
                    nc.scalar.mul(
                        ob[:, fc * CH : (fc + 1) * CH],
                        ops,
                        rl_sb[:, 2 * isub : 2 * isub + 1],
                    )
                nc.sync.dma_start(
                    out=out[c * CH + isub * P : c * CH + (isub + 1) * P, :],
                    in_=ob,
                )

            def attn(r, pending):
                """Attention for i-chunk r (transposed-scores softmax).
                pending = (c, attn_sb, rl_sb) of chunk c=r-1 whose output
                projection is interleaved at pipeline steps 1..4."""
                if mode == "causal":
                    # diagonal (masked) tiles first: their extra DVE mask
                    # latency hides in the 2-deep score-pipeline prologue
                    js = list(range(ND * r, ND * r + ND)) + list(range(ND * r))
                else:
                    js = list(range(NJT))
                jmax = len(js)
                ics = slice(r * CH, (r + 1) * CH)
                attn_ps = psp.tile([P, HD // P, CH], f32, tag="big", bufs=1, name="at")
                l_ps = psp.tile([4, CH], f32, tag="l", bufs=1, name="l")

                def mk_p(j):
                    stp = psp.tile([P, CH], f32, tag="st", bufs=3, name="stp")
                    for dt in range(HD // P):
                        nc.tensor.matmul(
                            stp,
                            krT_sb[:, dt, j * P : (j + 1) * P],
                            qrT_sb[:, dt, ics],
                            start=(dt == 0),
                            stop=(dt == HD // P - 1),
                        )
                    p = pw.tile([P, CH], FP16, tag="p", bufs=3, name="p")
                    nc.scalar.activation(p, stp, Exp)
                    if mode == "causal" and j >= ND * r:
                        nc.vector.tensor_mul(p, p, mk_sb[:, j - ND * r, :])
                    elif mode == "generic":
                        me = pw.tile([P, CH], BF16, tag="me", bufs=3, name="me")
                        nc.sync.dma_start(out=me, in_=mke[:, j, ics])
                        nc.vector.tensor_mul(p, p, me)
                    return p

                pq = deque()
                pq.append(mk_p(js[0]))
                pq.append(mk_p(js[1]))
                for t in range(jmax):
                    if t + 2 < jmax:
                        pq.append(mk_p(js[t + 2]))
                    p = pq.popleft()
                    j = js[t]
                    first, last = t == 0, t == jmax - 1
                    for et in range(HD // P):
                        nc.tensor.matmul(
                            attn_ps[:, et, :],
                            v_sb[:, j, et * P : (et + 1) * P],
                            p,
                            start=first,
                            stop=last,
                        )
                    nc.tensor.matmul(
                        l_ps, ones_sb[:, 0:4], p, start=first, stop=last
                    )
                    if pending is not None and 1 <= t <= ND:
                        fin_isub(pending[0], pending[1], pending[2], t - 1)
                # drain psums (frees banks for the next chunk)
                attn_sb = aw.tile([P, HD // P, CH], FP16, tag="at", bufs=2, name="atsb")
                nc.scalar.copy(attn_sb, attn_ps)
                l_sb = lw.tile([1, CH], F32R, tag="l", bufs=2, name="lsb")
                nc.vector.tensor_copy(l_sb, l_ps[0:1, :])
                return (r, attn_sb, l_sb)

            # ---------------- fused main pipeline ----------------
            # Causal: attn(r) only needs k/v chunks <= r, so proj(r+1) can be
            # interleaved after attn(r-1). Full attention (nomask/generic):
            # attn(0) reads ALL k/v chunks, so every projection must be
            # emitted first (still one continuous PE stream).
            proj(0)
            if mode != "causal":
                for c in range(1, NSC):
                    xcs.append(load_xc(c + 1) if c + 1 < NSC else None)
                    proj(c)
                # reorder: load_xc must come before proj uses it
            drained = None
            for r in range(NSC):
                if mode == "causal" and r + 1 < NSC:
                    proj(r + 1)
                    if r + 2 < NSC:
                        xcs.append(load_xc(r + 2))
                pend = None
                if drained is not None:
                    rl_sb = fin_setup(drained[2])
                    pend = (drained[0], drained[1], rl_sb)
                drained = attn(r, pend)
            rl_sb = fin_setup(drained[2])
            for isub in range(ND):
                fin_isub(drained[0], drained[1], rl_sb, isub)

    nc.compile()
    return nc


def _perm():
    return np.concatenate([np.arange(0, HD, 2), np.arange(1, HD, 2)])


def make_core_inputs(hidden_states, freqs_real, freqs_imag, mask, W_qkv, W_o, mode):
    """Host-side shard + relayout (free). Returns 8 in_maps (core = b*NH + h)."""
    perm = _perm()
    frT = np.ascontiguousarray(freqs_real.T.astype(np.float32))
    fiT = np.ascontiguousarray(freqs_imag.T.astype(np.float32))
    extras = {}
    if mode == "causal":
        r = np.arange(P)[:, None, None]
        o = np.arange(ND)[None, :, None]
        cc = np.arange(CH)[None, None, :]
        extras["mk"] = np.ascontiguousarray((cc >= r + P * o).astype(np.float16))
    elif mode == "generic":
        # exp(mask^T)[j, i] reshaped to [p, jt, i]
        m = np.exp(np.asarray(mask[0, 0], dtype=np.float64).T)  # [j, i]
        m = m.reshape(NJT, P, S).transpose(1, 0, 2)
        extras["mke"] = np.ascontiguousarray(m).astype(NPBF16)
    in_maps = []
    for b in range(B):
        xTr = np.ascontiguousarray(
            hidden_states[b].T.astype(np.float16).reshape(KT, P, S)
        )
        for h in range(NH):
            wq_h = (W_qkv[h * HD : (h + 1) * HD, :][perm, :] * SCALE).T
            wk_h = W_qkv[HID + h * HD : HID + (h + 1) * HD, :][perm, :].T
            wv_h = W_qkv[2 * HID + h * HD : 2 * HID + (h + 1) * HD, :].T
            wo_h = W_o[:, h * HD : (h + 1) * HD].T
            in_maps.append(
                {
                    "xT": xTr,
                    "wq": np.ascontiguousarray(
                        wq_h.reshape(KT, P, HD).transpose(1, 0, 2).astype(np.float16)
                    ),
                    "wk": np.ascontiguousarray(
                        wk_h.reshape(KT, P, HD).transpose(1, 0, 2).astype(np.float16)
                    ),
                    "wv": np.ascontiguousarray(
                        wv_h.reshape(KT, P, HD).transpose(1, 0, 2).astype(np.float16)
                    ),
                    "wo": np.ascontiguousarray(
                        wo_h.reshape(HD // P, P, HID)
                        .transpose(1, 0, 2)
                        .astype(np.float16)
                    ),
                    "frT": frT,
                    "fiT": fiT,
                    "ones": np.ones((P, 4), dtype=np.float16),
                    "ones2": np.ones((1, 2), dtype=np.float32),
                    **extras,
                }
            )
    return in_maps


def _mask_mode(mask):
    m = np.asarray(mask)
    if m.shape != (1, 1, S, S):
        return "generic"
    if not np.any(m):
        return "nomask"
    causal = np.tril(np.ones((S, S), dtype=bool))
    expect = np.where(causal, np.float32(0.0), np.float32(-1e9))
    if np.array_equal(m[0, 0], expect):
        return "causal"
    return "generic"


def kernel(hidden_states, freqs_real, freqs_imag, mask, W_qkv, W_o, _trace=False):
    hidden_states = np.asarray(hidden_states, dtype=np.float32)
    freqs_real = np.asarray(freqs_real, dtype=np.float32)
    freqs_imag = np.asarray(freqs_imag, dtype=np.float32)
    mask = np.asarray(mask)
    W_qkv = np.asarray(W_qkv, dtype=np.float32)
    W_o = np.asarray(W_o, dtype=np.float32)

    if _trace:
        _ensure_ntff_hook()
    mode = _mask_mode(mask)
    if mode not in _cache:
        _cache[mode] = build_nc(mode)
    nc = _cache[mode]
    in_maps = make_core_inputs(
        hidden_states, freqs_real, freqs_imag, mask, W_qkv, W_o, mode
    )
    res = run_bass_kernel_spmd(nc, in_maps, list(range(B * NH)), trace=_trace)
    outs = [res.results[i]["out"] for i in range(B * NH)]
    full = np.zeros((B, S, HID), dtype=np.float32)
    for b in range(B):
        for h in range(NH):
            full[b] += outs[b * NH + h]
    if _trace:
        return full, res
    return full


# revision 10
# speedup vs baseline: 1.3493x; 1.2131x over previous
"""Trainium2 Bass kernel for GemmaAttention (B=2, S=2048, HID=1024, NH=4, HD=256).

Sharding: 8 cores = batch(2) x heads(4). Each core computes one (b, h):
  q/k/v projections for its head, RoPE, attention, and a partial output
  projection [S, HID]; the host sums the 4 per-head partials per batch.

Design notes (v3 — fused pipeline, f32r):
  - Single fused PE stream: projections, attention and the output projection
    interleave so the tensor engine never idles (idle resets the PE p-state
    from 2.4GHz back to 1.2GHz). For causal masks proj(r+1) is interleaved
    after attn(r-1); for full attention all projections are emitted first.
  - All matmul operands are float32r: fp32 data read at full PE rate when the
    moving free dim >= 256 (FP22 multiply, fp32 accumulate) — same speed as
    bf16 at these tile shapes, ~10x better precision.
  - Scores are computed transposed, ST[j, i] = (q_i . k_j): exp needs no max
    subtraction, l[i] = sum_j P[j,i] is a ones-stationary matmul, and P^T is
    exactly the PV moving operand (no transposes anywhere).
  - Mask modes compiled on demand:
      nomask : mask == 0 -> full attention, no mask work at all
      causal : standard tril(-1e9) mask -> lower-triangle tiles only, binary
               multiplicative mask on diagonal tiles after exp; diagonal
               tiles are issued FIRST per chunk so the extra DVE mask latency
               hides in the score-pipeline prologue
      generic: arbitrary additive mask -> host precomputes exp(mask^T) in
               bf16; streamed in and applied multiplicatively after exp
  - Engine placement: exp / psum drains / (1/l) output scaling on ScalarE,
    RoPE on VectorE (ordered to release PSUM banks early), matmuls on PE.
"""

import sys

sys.path.insert(0, "/opt/trn_rl_repo")

from collections import deque

import numpy as np
import ml_dtypes

import concourse.bacc as bacc
import concourse.bass as bass
import concourse.mybir as mybir
import concourse.tile as tile
from concourse.bass_utils import run_bass_kernel_spmd


def _ensure_ntff_hook():
    """This image's ``antenv`` lacks ``axon_hooks`` (bass_utils imports it for
    trace=True). Inject an equivalent module driving NTFF profiling via the
    libaxon C ABI (mirrors trn_agent_boot._ntff_profile_via_ctypes)."""
    import types, ctypes, contextlib, os

    if "antenv.axon_hooks" in sys.modules:
        return
    so_path = "/opt/axon/libaxon_pjrt.so"
    hook = None
    if os.path.exists(so_path):
        lib = ctypes.CDLL(so_path)
        if hasattr(lib, "axon_start_nrt_profile"):
            lib.axon_start_nrt_profile.argtypes = [
                ctypes.POINTER(ctypes.c_int64),
                ctypes.c_size_t,
            ]
            lib.axon_start_nrt_profile.restype = ctypes.c_int64
            lib.axon_stop_nrt_profile.argtypes = [ctypes.c_char_p]
            lib.axon_stop_nrt_profile.restype = ctypes.c_int64

            @contextlib.contextmanager
            def _hook(output_dir, device_ids):
                import jax

                jax.devices()
                if device_ids:
                    ids = (ctypes.c_int64 * len(device_ids))(*device_ids)
                    rc = lib.axon_start_nrt_profile(ids, len(device_ids))
                else:
                    rc = lib.axon_start_nrt_profile(None, 0)
                if rc != 0:
                    raise RuntimeError(f"axon_start_nrt_profile rc={rc}")
                try:
                    yield
                finally:
                    n = lib.axon_stop_nrt_profile(str(output_dir).encode())
                    if n < 0:
                        raise RuntimeError(f"axon_stop_nrt_profile rc={n}")
                    print(f"profile: {n} file(s) written to {output_dir}")

            hook = _hook

    mod = types.ModuleType("antenv.axon_hooks")
    _state = {"hook": hook}
    mod.set_axon_ntff_profile_hook = lambda h: _state.__setitem__("hook", h)
    mod.get_axon_ntff_profile_hook = lambda: _state["hook"]
    sys.modules["antenv.axon_hooks"] = mod
    import antenv

    antenv.axon_hooks = mod


B, S, HID = 2, 2048, 1024
NH, HD = 4, 256
SCALE = HD**-0.5
P = 128
CH = 512          # i-chunk width (and matmul moving free-dim)
NSC = S // CH     # 4 i-chunks
NJT = S // P      # 16 j-tiles
KT = HID // P     # 8 contraction tiles for projections
ND = CH // P      # 4 i-subtiles per chunk

_cache = {}
F32R = mybir.dt.float32r
FP16 = mybir.dt.float16
BF16 = mybir.dt.bfloat16
NPBF16 = ml_dtypes.bfloat16


def build_nc(mode):
    """Emit the single-core program (SPMD: all 8 cores run this)."""
    assert mode in ("nomask", "causal", "generic")
    nc = bacc.Bacc()
    f32 = mybir.dt.float32
    Exp = mybir.ActivationFunctionType.Exp

    xT = nc.declare_dram_parameter("xT", [KT, P, S], FP16, isOutput=False)
    wq = nc.declare_dram_parameter("wq", [P, KT, HD], FP16, isOutput=False)
    wk = nc.declare_dram_parameter("wk", [P, KT, HD], FP16, isOutput=False)
    wv = nc.declare_dram_parameter("wv", [P, KT, HD], FP16, isOutput=False)
    wo = nc.declare_dram_parameter("wo", [P, HD // P, HID], FP16, isOutput=False)
    frT = nc.declare_dram_parameter("frT", [P, S], f32, isOutput=False)
    fiT = nc.declare_dram_parameter("fiT", [P, S], f32, isOutput=False)
    ones = nc.declare_dram_parameter("ones", [P, 4], F32R, isOutput=False)
    ones2 = nc.declare_dram_parameter("ones2", [1, 2], F32R, isOutput=False)
    if mode == "causal":
        mk = nc.declare_dram_parameter("mk", [P, ND, CH], FP16, isOutput=False)
    elif mode == "generic":
        mke = nc.declare_dram_parameter("mke", [P, NJT, S], BF16, isOutput=False)
    out = nc.declare_dram_parameter("out", [S, HID], f32, isOutput=True)

    with tile.TileContext(nc) as tc:
        with (
            tc.tile_pool(name="consts", bufs=1) as consts,
            tc.tile_pool(name="xp", bufs=2) as xp,
            tc.tile_pool(name="qk", bufs=1) as qk,
            tc.tile_pool(name="rst", bufs=2) as rst,
            tc.tile_pool(name="pw", bufs=3) as pw,
            tc.tile_pool(name="aw", bufs=2) as aw,
            tc.tile_pool(name="lw", bufs=2) as lw,
            tc.tile_pool(name="obp", bufs=3) as obp,
            tc.tile_pool(name="psp", bufs=1, space="PSUM") as psp,
        ):
            # ---------------- constant + input loads ----------------
            # Order matters: the first projection matmul only needs wq[kt=0]
            # and xc0[kt=0], so weights are loaded per-kt interleaved with the
            # xT chunk tiles, and freqs per-chunk, to start the PE ASAP.
            wq_sb = consts.tile([P, KT, HD], FP16)
            wk_sb = consts.tile([P, KT, HD], FP16)
            frT_sb = consts.tile([P, S], f32)
            fiT_sb = consts.tile([P, S], f32)

            # bulk stream (weights + x chunks + stores) on the sync HWDGE
            # ring; small constants (freqs/mask/ones) on the independent
            # scalar-engine HWDGE ring so they don't queue behind the bulk.
            def load_xc(c):
                xc = xp.tile([P, KT, CH], FP16, tag="xc", name="xc")
                cs = slice(c * CH, (c + 1) * CH)
                for k0 in (0, KT // 2):
                    nc.sync.dma_start(
                        out=xc[:, k0 : k0 + KT // 2, :],
                        in_=xT[k0 : k0 + KT // 2, :, cs].rearrange("k p s -> p k s"),
                    )
                return xc

            nc.sync.dma_start(out=wq_sb, in_=wq[:])
            xc0 = load_xc(0)
            nc.sync.dma_start(out=wk_sb, in_=wk[:])
            nc.scalar.dma_start(out=frT_sb, in_=frT[:])
            nc.scalar.dma_start(out=fiT_sb, in_=fiT[:])
            wv_sb = consts.tile([P, KT, HD], FP16)
            nc.sync.dma_start(out=wv_sb, in_=wv[:])
            xcs = [xc0, load_xc(1)]
            wo_sb = consts.tile([P, HD // P, HID], FP16)
            nc.sync.dma_start(out=wo_sb, in_=wo[:])
            if mode == "causal":
                mk_sb = consts.tile([P, ND, CH], FP16)
                nc.scalar.dma_start(out=mk_sb, in_=mk[:])
            ones_sb = consts.tile([P, 4], F32R)
            nc.scalar.dma_start(out=ones_sb, in_=ones[:])
            ones2_sb = consts.tile([1, 2], F32R)
            nc.scalar.dma_start(out=ones2_sb, in_=ones2[:])

            # persistent activations
            qrT_sb = qk.tile([P, HD // P, S], FP16)   # rope'd qT (d on partitions)
            krT_sb = qk.tile([P, HD // P, S], FP16)
            v_sb = qk.tile([P, NJT, HD], FP16)        # v[j, e] per j-tile

            # ---------------- building blocks ----------------
            def proj(c):
                """q/k projections + fused rope for i-chunk c; v for its 4
                i-tiles. RoPE op order releases ps0 after 2 DVE ops so the
                "st" psum ring (3 bufs) never stalls the next projection."""
                xc = xcs[c]
                cs = slice(c * CH, (c + 1) * CH)
                for wsb, dst in ((wq_sb, qrT_sb), (wk_sb, krT_sb)):
                    ps0 = psp.tile([P, CH], f32, tag="st", bufs=4, name="ps0")
                    ps1 = psp.tile([P, CH], f32, tag="st", bufs=4, name="ps1")
                    for m, ps in ((0, ps0), (1, ps1)):
                        for kt in range(KT):
                            nc.tensor.matmul(
                                ps,
                                wsb[:, kt, m * P : (m + 1) * P],
                                xc[:, kt, :],
                                start=(kt == 0),
                                stop=(kt == KT - 1),
                            )
                    fr = frT_sb[:, cs]
                    fi = fiT_sb[:, cs]
                    ta = rst.tile([P, CH], f32, tag="ta", bufs=2, name="ta")
                    tb = rst.tile([P, CH], f32, tag="tb", bufs=2, name="tb")
                    # dst0 = ps0*fr - ps1*fi ; dst1 = ps1*fr + ps0*fi
                    nc.vector.tensor_mul(dst[:, 0, cs], ps0, fr)
                    nc.vector.tensor_mul(tb, ps0, fi)
                    nc.vector.tensor_mul(ta, ps1, fi)
                    nc.vector.tensor_mul(dst[:, 1, cs], ps1, fr)
                    nc.vector.tensor_sub(dst[:, 0, cs], dst[:, 0, cs], ta)
                    nc.vector.tensor_add(dst[:, 1, cs], dst[:, 1, cs], tb)
                for sl in range(ND):
                    st = ND * c + sl
                    psv = psp.tile([P, HD], f32, tag="o", bufs=2, name="psv")
                    for kt in range(KT):
                        nc.tensor.matmul(
                            psv,
                            xc[:, kt, sl * P : (sl + 1) * P],
                            wv_sb[:, kt, :],
                            start=(kt == 0),
                            stop=(kt == KT - 1),
                        )
                    nc.scalar.copy(v_sb[:, st, :], psv)

            def fin_setup(acc):
                """l = ones^T @ acc (one matmul per chunk), transpose l to
                partitions (tiny matmuls) + reciprocal."""
                l_ps = psp.tile([4, CH], f32, tag="o", bufs=2, name="lps")
                nc.tensor.matmul(l_ps, ones_sb, acc, start=True, stop=True)
                l_sb = lw.tile([1, CH], F32R, tag="l", bufs=2, name="lsb")
                nc.vector.tensor_copy(l_sb, l_ps[0:1, :])
                rl_ps = psp.tile([P, 2 * ND], f32, tag="o", bufs=2, name="rlps")
                for i in range(ND):
                    nc.tensor.matmul(
                        rl_ps[:, 2 * i : 2 * i + 2],
                        l_sb[:, i * P : (i + 1) * P],
                        ones2_sb,
                        start=True,
                        stop=True,
                    )
                rl_sb = lw.tile([P, 2 * ND], f32, tag="rl", bufs=2, name="rlsb")
                nc.vector.reciprocal(rl_sb, rl_ps)
                return rl_sb

            def fin_isub(c, attn_sb, rl_sb, isub):
                """Output projection + 1/l scaling + store for one i-subtile."""
                ob = obp.tile([P, HID], f32, tag="ob", bufs=3, name="ob")
                for fc in range(HID // CH):
                    ops = psp.tile([P, CH], f32, tag="o", bufs=2, name="ops")
                    for et in range(HD // P):
                        nc.tensor.matmul(
                            ops,
                            attn_sb[:, et, isub * P : (isub + 1) * P],
                            wo_sb[:, et, fc * CH : (fc + 1) * CH],
                            start=(et == 0),
                            stop=(et == HD // P - 1),
                        )
                    nc.scalar.mul(
                        ob[:, fc * CH : (fc + 1) * CH],
                        ops,
                        rl_sb[:, 2 * isub : 2 * isub + 1],
                    )
                nc.sync.dma_start(
                    out=out[c * CH + isub * P : c * CH + (isub + 1) * P, :],
                    in_=ob,
                )

            def attn(r, pending):
                """Attention for i-chunk r (transposed-scores softmax).
                pending = (c, attn_sb, rl_sb) of chunk c=r-1 whose output
                projection is interleaved at pipeline steps 1..4."""
                if mode == "causal":
                    # diagonal (masked) tiles first: their extra DVE mask
                    # latency hides in the 2-deep score-pipeline prologue
                    js = list(range(ND * r, ND * r + ND)) + list(range(ND * r))
                else:
                    js = list(range(NJT))
                jmax = len(js)
                ics = slice(r * CH, (r + 1) * CH)
                attn_ps = psp.tile([P, HD // P, CH], f32, tag="big", bufs=1, name="at")
                acc = lw.tile([P, CH], F32R, tag="acc", bufs=2, name="acc")

                def mk_p(j):
                    stp = psp.tile([P, CH], f32, tag="st", bufs=4, name="stp")
                    for dt in range(HD // P):
                        nc.tensor.matmul(
                            stp,
                            krT_sb[:, dt, j * P : (j + 1) * P],
                            qrT_sb[:, dt, ics],
                            start=(dt == 0),
                            stop=(dt == HD // P - 1),
                        )
                    p = pw.tile([P, CH], FP16, tag="p", bufs=4, name="p")
                    nc.scalar.activation(p, stp, Exp)
                    if mode == "causal" and j >= ND * r:
                        nc.vector.tensor_mul(p, p, mk_sb[:, j - ND * r, :])
                    elif mode == "generic":
                        me = pw.tile([P, CH], BF16, tag="me", bufs=3, name="me")
                        nc.sync.dma_start(out=me, in_=mke[:, j, ics])
                        nc.vector.tensor_mul(p, p, me)
                    return p

                pq = deque()
                pq.append(mk_p(js[0]))
                pq.append(mk_p(js[1]))
                pq.append(mk_p(js[2]))
                rl_sb = fin_setup(pending[2]) if pending is not None else None
                for t in range(jmax):
                    if t + 3 < jmax:
                        pq.append(mk_p(js[t + 3]))
                    p = pq.popleft()
                    j = js[t]
                    first, last = t == 0, t == jmax - 1
                    for et in range(HD // P):
                        nc.tensor.matmul(
                            attn_ps[:, et, :],
                            v_sb[:, j, et * P : (et + 1) * P],
                            p,
                            start=first,
                            stop=last,
                        )
                    if first:
                        nc.vector.tensor_copy(acc, p)
                    else:
                        nc.vector.tensor_add(acc, acc, p)
                    if pending is not None and 1 <= t <= ND:
                        fin_isub(pending[0], pending[1], rl_sb, t - 1)
                # drain psum (frees banks for the next chunk)
                attn_sb = aw.tile([P, HD // P, CH], FP16, tag="at", bufs=2, name="atsb")
                nc.scalar.copy(attn_sb, attn_ps)
                return (r, attn_sb, acc)

            # ---------------- fused main pipeline ----------------
            # Causal: attn(r) only needs k/v chunks <= r, so proj(r+1) can be
            # interleaved after attn(r-1). Full attention (nomask/generic):
            # attn(0) reads ALL k/v chunks, so every projection must be
            # emitted first (still one continuous PE stream).
            proj(0)
            if mode != "causal":
                for c in range(1, NSC):
                    xcs.append(load_xc(c + 1) if c + 1 < NSC else None)
                    proj(c)
                # reorder: load_xc must come before proj uses it
            drained = None
            for r in range(NSC):
                if mode == "causal" and r + 1 < NSC:
                    proj(r + 1)
                    if r + 2 < NSC:
                        xcs.append(load_xc(r + 2))
                drained = attn(r, drained)
            rl_sb = fin_setup(drained[2])
            for isub in range(ND):
                fin_isub(drained[0], drained[1], rl_sb, isub)

    nc.compile()
    return nc


def _perm():
    return np.concatenate([np.arange(0, HD, 2), np.arange(1, HD, 2)])


def make_core_inputs(hidden_states, freqs_real, freqs_imag, mask, W_qkv, W_o, mode):
    """Host-side shard + relayout (free). Returns 8 in_maps (core = b*NH + h)."""
    perm = _perm()
    frT = np.ascontiguousarray(freqs_real.T.astype(np.float32))
    fiT = np.ascontiguousarray(freqs_imag.T.astype(np.float32))
    extras = {}
    if mode == "causal":
        r = np.arange(P)[:, None, None]
        o = np.arange(ND)[None, :, None]
        cc = np.arange(CH)[None, None, :]
        extras["mk"] = np.ascontiguousarray((cc >= r + P * o).astype(np.float16))
    elif mode == "generic":
        # exp(mask^T)[j, i] reshaped to [p, jt, i]
        m = np.exp(np.asarray(mask[0, 0], dtype=np.float64).T)  # [j, i]
        m = m.reshape(NJT, P, S).transpose(1, 0, 2)
        extras["mke"] = np.ascontiguousarray(m).astype(NPBF16)
    in_maps = []
    for b in range(B):
        xTr = np.ascontiguousarray(
            hidden_states[b].T.astype(np.float16).reshape(KT, P, S)
        )
        for h in range(NH):
            wq_h = (W_qkv[h * HD : (h + 1) * HD, :][perm, :] * SCALE).T
            wk_h = W_qkv[HID + h * HD : HID + (h + 1) * HD, :][perm, :].T
            wv_h = W_qkv[2 * HID + h * HD : 2 * HID + (h + 1) * HD, :].T
            wo_h = W_o[:, h * HD : (h + 1) * HD].T
            in_maps.append(
                {
                    "xT": xTr,
                    "wq": np.ascontiguousarray(
                        wq_h.reshape(KT, P, HD).transpose(1, 0, 2).astype(np.float16)
                    ),
                    "wk": np.ascontiguousarray(
                        wk_h.reshape(KT, P, HD).transpose(1, 0, 2).astype(np.float16)
                    ),
                    "wv": np.ascontiguousarray(
                        wv_h.reshape(KT, P, HD).transpose(1, 0, 2).astype(np.float16)
                    ),
                    "wo": np.ascontiguousarray(
                        wo_h.reshape(HD // P, P, HID)
                        .transpose(1, 0, 2)
                        .astype(np.float16)
                    ),
                    "frT": frT,
                    "fiT": fiT,
                    "ones": np.ones((P, 4), dtype=np.float32),
                    "ones2": np.ones((1, 2), dtype=np.float32),
                    **extras,
                }
            )
    return in_maps


def _mask_mode(mask):
    m = np.asarray(mask)
    if m.shape != (1, 1, S, S):
        return "generic"
    if not np.any(m):
        return "nomask"
    causal = np.tril(np.ones((S, S), dtype=bool))
    expect = np.where(causal, np.float32(0.0), np.float32(-1e9))
    if np.array_equal(m[0, 0], expect):
        return "causal"
    return "generic"


def kernel(hidden_states, freqs_real, freqs_imag, mask, W_qkv, W_o, _trace=False):
    hidden_states = np.asarray(hidden_states, dtype=np.float32)
    freqs_real = np.asarray(freqs_real, dtype=np.float32)
    freqs_imag = np.asarray(freqs_imag, dtype=np.float32)
    mask = np.asarray(mask)
    W_qkv = np.asarray(W_qkv, dtype=np.float32)
    W_o = np.asarray(W_o, dtype=np.float32)

    if _trace:
        _ensure_ntff_hook()
    mode = _mask_mode(mask)
    if mode not in _cache:
        _cache[mode] = build_nc(mode)
    nc = _cache[mode]
    in_maps = make_core_inputs(
        hidden_states, freqs_real, freqs_imag, mask, W_qkv, W_o, mode
    )
    res = run_bass_kernel_spmd(nc, in_maps, list(range(B * NH)), trace=_trace)
    outs = [res.results[i]["out"] for i in range(B * NH)]
    full = np.zeros((B, S, HID), dtype=np.float32)
    for b in range(B):
        for h in range(NH):
            full[b] += outs[b * NH + h]
    if _trace:
        return full, res
    return full
